# revision 26
# baseline (speedup 1.0000x reference)
"""Trainium2 Bass kernel for nn_ContextDrivingForce (dense MLP, 3 fused layers).

Math (per token row, D=896):
    u_proj = u @ W_a.T + b_a
    alpha  = sigmoid(sum(h * u_proj) / sqrt(D))
    u_att  = alpha * u
    g      = sigmoid([h, u_att] @ W_g.T + b_g)
    u_gate = g * u_att
    out    = gelu([h, u_gate, h*u_gate] @ W_f.T + b_f)        (exact erf gelu)

Distribution: data-parallel over the token axis across 8 NeuronCores,
weights replicated. All device tensors are feature-major ([D, tokens]);
the host transposes inputs/weights and the final output, so the device
performs no transposes at all.

fp8 mode (default): layers 1 and 2 run as float8e4 DoubleRow matmuls
(2 k-tiles of 128 contracted per instruction at 0.5 cycles/row), layer 3
stays bf16 for accuracy.  Layer 2 is restructured as
    z2 = Wg1 @ h + alpha * (Wg2 @ u)
so its matmuls depend only on the (host-quantized) fp8 inputs, not on
layer 1's output; alpha is applied to the PSUM on the vector engine.
Layer 1's bias rides a padded weight row against a constant-SX lane in
the fp8 input tile.  Sigmoids are computed via tanh so every activation
lives in the single `gelu_and_others` ACT table (no table reloads):
    alpha = (1 + tanh(logit/2)) / 2
    ug''  = (1 + tanh((z2+bg)/2)) * alpha * u = 2*u_gate  -> W_f[:,D:] *= 1/2
"""

import math
import sys
from contextlib import ExitStack

for _p in ("/root/.axon_site", "/root/.axon_site/_ro/trn_rl_repo"):
    if _p not in sys.path:
        sys.path.append(_p)

import ml_dtypes
import numpy as np

import concourse.bass as bass
import concourse.mybir as mybir
import concourse.tile as tile
from concourse import bacc
from concourse.bass_utils import run_bass_kernel_spmd

P = 128
D = 896
KD = D // P  # 7 feature tiles
N_TOK = 16384
N_CORES = 8
NPC = N_TOK // N_CORES  # 2048 tokens per core

F32 = mybir.dt.float32
BF16 = mybir.dt.bfloat16
F8 = mybir.dt.float8e4
AF = mybir.ActivationFunctionType
ALU = mybir.AluOpType
DR = mybir.MatmulPerfMode.DoubleRow

# fp8 quantization scales (host folds the inverses into PSUM dequant)
SX = 16.0    # inputs h, u
SWA = 512.0  # W_a
SWG = 512.0  # W_g
C1 = 1.0 / (SX * SWA)
CG = 0.5 / (SX * SWG)  # includes the tanh-trick 1/2


def build_nc_fp8(npc=NPC, T=512, mm_bufs=4, mm3_bufs=2, act_bufs=2):
    """fp8 DoubleRow program: L1+L2 fp8, L3 bf16.

    Software-pipelined: iteration c emits chunk c's L1/L2/epilogue and chunk
    c-1's L3 interleaved, so every cross-engine dependency (logit tree, alpha
    chain, ug/hu production) has a multi-microsecond window of independent
    matmul work in front of it, and the W_f weight DMA hides behind two
    chunks of fp8 L1/L2 work at startup.
    """
    n_chunks = npc // T
    assert n_chunks * T == npc
    inv_sqrt_d = 1.0 / math.sqrt(D)

    nc = bacc.Bacc()
    # xu8 groups: 0..6 = h k-tiles * SX, 7 = zeros, 8..14 = u k-tiles * SX,
    # 15 = zeros except partition 0 = SX (constant lane for the L1 bias row)
    xu8_d = nc.declare_dram_parameter("xu8", [P, n_chunks, 16, T], F8, isOutput=False)
    h16_d = nc.declare_dram_parameter("h16", [P, n_chunks, KD, T], BF16, isOutput=False)
    u16_d = nc.declare_dram_parameter("u16", [P, n_chunks, KD, T], BF16, isOutput=False)
    # fp8 weights packed as DoubleRow pairs: [P, pair, 2, D_out]
    wa_d = nc.declare_dram_parameter("wa8", [P, 4, 2, D], F8, isOutput=False)
    wga_d = nc.declare_dram_parameter("wga8", [P, 4, 2, D], F8, isOutput=False)
    wgb_d = nc.declare_dram_parameter("wgb8", [P, 4, 2, D], F8, isOutput=False)
    wf_d = nc.declare_dram_parameter("wf16", [P, 3 * KD, D], BF16, isOutput=False)
    bias_d = nc.declare_dram_parameter("biasp", [P, 2 * KD], F32, isOutput=False)
    gT_d = nc.declare_dram_parameter("gT", [D, npc], BF16, isOutput=True)

    with tile.TileContext(nc) as tc, ExitStack() as ctx:
        wp = ctx.enter_context(tc.tile_pool(name="weights", bufs=1))
        xup = ctx.enter_context(tc.tile_pool(name="xup", bufs=act_bufs))
        hp = ctx.enter_context(tc.tile_pool(name="hp", bufs=act_bufs))
        up = ctx.enter_context(tc.tile_pool(name="up", bufs=act_bufs))
        tmpp = ctx.enter_context(tc.tile_pool(name="tmpp", bufs=act_bufs))
        ugp = ctx.enter_context(tc.tile_pool(name="ugp", bufs=act_bufs))
        hup = ctx.enter_context(tc.tile_pool(name="hup", bufs=act_bufs))
        sp = ctx.enter_context(tc.tile_pool(name="small", bufs=3))
        rp = ctx.enter_context(tc.tile_pool(name="rows", bufs=2))
        abp = ctx.enter_context(tc.tile_pool(name="abp", bufs=2))
        op = ctx.enter_context(tc.tile_pool(name="outp", bufs=3))
        pp = ctx.enter_context(tc.tile_pool(name="psum", bufs=1, space="PSUM"))

        bias_sb = wp.tile([P, 2 * KD], F32, name="biasp")
        ones_col = wp.tile([P, 1], BF16, name="ones_col")
        nc.vector.memset(ones_col, 1.0)
        ones_row = wp.tile([1, P], BF16, name="ones_row")
        nc.vector.memset(ones_row, 1.0)

        wa_sb = wp.tile([P, 4, 2, D], F8, name="wa8")
        wga_sb = wp.tile([P, 4, 2, D], F8, name="wga8")
        wgb_sb = wp.tile([P, 4, 2, D], F8, name="wgb8")
        wf_sb = wp.tile([P, 3 * KD, D], BF16, name="wf16")

        def load_chunk(c):
            xu = xup.tile([P, 16, T], F8, name=f"xu{c}", tag="xu")
            nc.sync.dma_start(xu, xu8_d[:, c])
            h_sb = hp.tile([P, KD, T], BF16, name=f"h{c}", tag="h")
            nc.sync.dma_start(h_sb, h16_d[:, c])
            u_sb = up.tile([P, KD, T], BF16, name=f"u{c}", tag="u")
            nc.sync.dma_start(u_sb, u16_d[:, c])
            return xu, h_sb, u_sb

        # Prelude, ordered by first consumption: W_a + u-side of xu0 (L1),
        # then W_g-B + h-side of xu0 (L2-B is emitted before L2-A), h16
        # (logit tmps), W_g-A, u16, chunk-1 inputs, then all of W_f (only
        # needed once chunk 0's delayed L3 starts, two L1/L2 phases in).
        xu0 = xup.tile([P, 16, T], F8, name="xu0", tag="xu")
        h0 = hp.tile([P, KD, T], BF16, name="h0", tag="h")
        u0 = up.tile([P, KD, T], BF16, name="u0", tag="u")
        # Every dma_start costs ~0.6us of serialized descriptor-issue time on
        # the sync queue, so the prelude uses few, large transfers ordered by
        # first consumption.
        # halves let compute start on partial weights/inputs
        nc.sync.dma_start(wa_sb[:, :2], wa_d[:, :2])
        nc.sync.dma_start(xu0[:, 8:12, :], xu8_d[:, 0, 8:12])
        nc.sync.dma_start(wa_sb[:, 2:], wa_d[:, 2:])
        nc.sync.dma_start(xu0[:, 12:, :], xu8_d[:, 0, 12:])
        nc.sync.dma_start(h0, h16_d[:, 0])                  # gates the c0 logit
        nc.sync.dma_start(wgb_sb[:, :2], wgb_d[:, :2])
        nc.sync.dma_start(wgb_sb[:, 2:], wgb_d[:, 2:])
        nc.sync.dma_start(xu0[:, :4, :], xu8_d[:, 0, :4])   # h-side (L2-A)
        nc.sync.dma_start(xu0[:, 4:8, :], xu8_d[:, 0, 4:8])
        nc.sync.dma_start(wga_sb[:, :2], wga_d[:, :2])
        nc.sync.dma_start(wga_sb[:, 2:], wga_d[:, 2:])
        chunks = {0: (xu0, h0, u0)}
        if n_chunks > 1:
            # weave chunk-1 inputs into the W_f stream by first need time:
            # xu1 before (L1(c1)), then wf k0..6 (L3(c0) h-channel), h1, ...
            xu1 = xup.tile([P, 16, T], F8, name="xu1", tag="xu")
            nc.sync.dma_start(xu1, xu8_d[:, 1])
            h1 = hp.tile([P, KD, T], BF16, name="h1", tag="h")
            u1 = up.tile([P, KD, T], BF16, name="u1", tag="u")
            chunks[1] = (xu1, h1, u1)
            # u0/bias deadlines (c0 epilogue on DVE/ACT) are later than xu1's
            # (tensor L1(c1)), so they ride after it
            nc.sync.dma_start(u0, u16_d[:, 0])
            nc.sync.dma_start(bias_sb, bias_d[:, :])
            nc.sync.dma_start(wf_sb[:, :KD], wf_d[:, :KD])
            nc.sync.dma_start(h1, h16_d[:, 1])
            nc.sync.dma_start(wf_sb[:, KD:2 * KD], wf_d[:, KD:2 * KD])
            nc.sync.dma_start(u1, u16_d[:, 1])
            nc.sync.dma_start(wf_sb[:, 2 * KD:], wf_d[:, 2 * KD:])
        else:
            nc.sync.dma_start(u0, u16_d[:, 0])
            nc.sync.dma_start(bias_sb, bias_d[:, :])
            nc.sync.dma_start(wf_sb, wf_d[:, :])

        state = {}  # c -> (h_sb, ug, hu) for the pipelined (delayed) L3

        def emit_l1(c, xu, h_sb):
            # ps1 = (Wa*SWA + bias row) @ (u*SX), 4 DR pairs per m
            tmps = tmpp.tile([P, KD, T], BF16, name=f"tmps{c}", tag="tmps")
            for m in range(KD):
                ps1 = pp.tile([P, T], F32, name=f"ps1_{c}_{m}", tag="mm",
                              bufs=mm_bufs)
                for j in range(4):
                    nc.tensor.matmul(ps1, lhsT=wa_sb[:, j, :, m * P:(m + 1) * P],
                                     rhs=xu[:, 8 + 2 * j:10 + 2 * j, :],
                                     start=(j == 0), stop=(j == 3), perf_mode=DR)
                # tmp_m = (u_proj + b_a) * h   (C1 dequant + fused multiply)
                nc.vector.scalar_tensor_tensor(
                    out=tmps[:, m, :], in0=ps1, scalar=C1,
                    in1=h_sb[:, m, :], op0=ALU.mult, op1=ALU.mult)
            # partition-reduce tree for the logit row (DVE)
            t01 = sp.tile([P, T], BF16, name=f"t01_{c}", tag="tr", bufs=6)
            nc.vector.tensor_add(out=t01, in0=tmps[:, 0, :], in1=tmps[:, 1, :])
            t23 = sp.tile([P, T], BF16, name=f"t23_{c}", tag="tr", bufs=6)
            nc.vector.tensor_add(out=t23, in0=tmps[:, 2, :], in1=tmps[:, 3, :])
            t45 = sp.tile([P, T], BF16, name=f"t45_{c}", tag="tr", bufs=6)
            nc.vector.tensor_add(out=t45, in0=tmps[:, 4, :], in1=tmps[:, 5, :])
            t03 = sp.tile([P, T], BF16, name=f"t03_{c}", tag="tr", bufs=6)
            nc.vector.tensor_add(out=t03, in0=t01, in1=t23)
            t46 = sp.tile([P, T], BF16, name=f"t46_{c}", tag="tr", bufs=6)
            nc.vector.tensor_add(out=t46, in0=t45, in1=tmps[:, 6, :])
            S = sp.tile([P, T], BF16, name=f"S_{c}", tag="tr", bufs=6)
            nc.vector.tensor_add(out=S, in0=t03, in1=t46)
            return S

        def emit_alpha(c, S):
            # alpha = 0.5*tanh(logit/2) + 0.5, broadcast via rank-1 matmul
            red = pp.tile([1, T], F32, name=f"red{c}", tag="red", bufs=1)
            nc.tensor.matmul(red, lhsT=ones_col, rhs=S, start=True, stop=True)
            t1row = rp.tile([1, T], BF16, name=f"t1_{c}", tag="t1")
            nc.scalar.activation(t1row, red, AF.Tanh, scale=inv_sqrt_d * 0.5)
            arow = rp.tile([1, T], BF16, name=f"ar_{c}", tag="ar")
            nc.vector.tensor_scalar(out=arow, in0=t1row, scalar1=1.0,
                                    scalar2=0.5, op0=ALU.add, op1=ALU.mult)
            ab = pp.tile([P, T], F32, name=f"ab{c}", tag="ab", bufs=1)
            nc.tensor.matmul(ab, lhsT=ones_row, rhs=arow, start=True, stop=True)
            ab_sb = abp.tile([P, T], BF16, name=f"absb{c}", tag="ab")
            nc.scalar.activation(ab_sb, ab, AF.Copy, scale=1.0)
            return ab_sb

        def l2mm(c, xu, m, psAs, psBs):
            psB = pp.tile([P, T], F32, name=f"psB{c}_{m}", tag="mm",
                          bufs=mm_bufs)
            for j in range(4):
                nc.tensor.matmul(psB, lhsT=wgb_sb[:, j, :, m * P:(m + 1) * P],
                                 rhs=xu[:, 8 + 2 * j:10 + 2 * j, :],
                                 start=(j == 0), stop=(j == 3), perf_mode=DR)
            psA = pp.tile([P, T], F32, name=f"psA{c}_{m}", tag="mm",
                          bufs=mm_bufs)
            for j in range(4):
                nc.tensor.matmul(psA, lhsT=wga_sb[:, j, :, m * P:(m + 1) * P],
                                 rhs=xu[:, 2 * j:2 * j + 2, :],
                                 start=(j == 0), stop=(j == 3), perf_mode=DR)
            psAs[m], psBs[m] = psA, psB

        def emit_epilogue(c, h_sb, u_sb, psAs, psBs, ab_sb):
            ug = ugp.tile([P, KD, T], BF16, name=f"ug{c}", tag="ug")
            hu = hup.tile([P, KD, T], BF16, name=f"hu{c}", tag="hu")
            for m in range(KD):
                q = sp.tile([P, T], BF16, name=f"q{c}_{m}", tag="q")
                nc.vector.tensor_mul(out=q, in0=psBs[m], in1=ab_sb)
                z2h = sp.tile([P, T], BF16, name=f"z2h{c}_{m}", tag="z2h")
                nc.vector.scalar_tensor_tensor(
                    out=z2h, in0=psAs[m], scalar=1.0, in1=q,
                    op0=ALU.mult, op1=ALU.add)
                t2 = sp.tile([P, T], BF16, name=f"t2_{c}_{m}", tag="t2")
                nc.scalar.activation(t2, z2h, AF.Tanh,
                                     bias=bias_sb[:, m:m + 1], scale=CG)
                w1 = sp.tile([P, T], BF16, name=f"w1_{c}_{m}", tag="w1")
                nc.vector.tensor_mul(out=w1, in0=ab_sb, in1=u_sb[:, m, :])
                # ug'' = (1 + tanh) * alpha * u = 2*u_gate
                nc.vector.scalar_tensor_tensor(
                    out=ug[:, m, :], in0=t2, scalar=1.0, in1=w1,
                    op0=ALU.add, op1=ALU.mult)
                nc.vector.tensor_mul(out=hu[:, m, :], in0=h_sb[:, m, :],
                                     in1=ug[:, m, :])
            return ug, hu

        def emit_l3(c, ms):
            h_sb, ug, hu = state[c]
            cs = bass.ds(c * T, T)
            for m in ms:
                ps3 = pp.tile([P, T], F32, name=f"ps3_{c}_{m}", tag="mm3",
                              bufs=mm3_bufs)
                for k in range(3 * KD):
                    if k < KD:
                        rhs = h_sb[:, k, :]
                    elif k < 2 * KD:
                        rhs = ug[:, k - KD, :]
                    else:
                        rhs = hu[:, k - 2 * KD, :]
                    nc.tensor.matmul(ps3, lhsT=wf_sb[:, k, m * P:(m + 1) * P],
                                     rhs=rhs, start=(k == 0),
                                     stop=(k == 3 * KD - 1))
                outp = op.tile([P, T], BF16, name=f"o{c}_{m}", tag="out")
                nc.scalar.activation(outp, ps3, AF.Gelu,
                                     bias=bias_sb[:, KD + m:KD + m + 1],
                                     scale=1.0)
                # output stores ride the ACT HWDGE ring
                nc.scalar.dma_start(gT_d[m * P:(m + 1) * P, cs], outp)

        for c in range(n_chunks):
            if c >= 2:
                chunks[c] = load_chunk(c)
            xu, h_sb, u_sb = chunks[c]
            psAs, psBs = {}, {}

            # fp8 work is emitted contiguously (L1 then L2); the delayed
            # bf16 L3 of chunk c-1 comes last. The PE pays ~190ns per
            # bf16->fp8 pumping-mode switch, so same-dtype runs matter; the
            # tile scheduler still pulls ready L3 matmuls into any genuine
            # fp8 stall hole (alpha chain) on its own.
            S = emit_l1(c, xu, h_sb)
            # two L2 m-blocks pre-emitted: 4 mm-tile allocations is the
            # bufs=4 limit before an alpha-chain consumer would deadlock
            l2mm(c, xu, 0, psAs, psBs)
            l2mm(c, xu, 1, psAs, psBs)
            ab_sb = emit_alpha(c, S)
            for m in range(KD):
                if m not in psAs:
                    l2mm(c, xu, m, psAs, psBs)
            ug, hu = emit_epilogue(c, h_sb, u_sb, psAs, psBs, ab_sb)
            state[c] = (h_sb, ug, hu)
            if c >= 1:
                emit_l3(c - 1, list(range(KD)))
        emit_l3(n_chunks - 1, list(range(KD)))
    nc.compile()
    return nc


def prep_inputs_fp8(h_t, u_t, W_a_w, W_a_b, W_g_w, W_g_b, W_f_w, W_f_b,
                    npc=NPC, T=512):
    F8NP = ml_dtypes.float8_e4m3
    BFNP = ml_dtypes.bfloat16
    h = np.asarray(h_t, np.float32)
    u = np.asarray(u_t, np.float32)
    Wa = np.asarray(W_a_w, np.float32)
    Wg = np.asarray(W_g_w, np.float32)
    Wf = np.asarray(W_f_w, np.float32)
    ba = np.asarray(W_a_b, np.float32)
    bg = np.asarray(W_g_b, np.float32)
    bf = np.asarray(W_f_b, np.float32)
    nch = npc // T

    def wpack8(wT_scaled, bias_row=None):
        # [896, D_out] (+optional bias row at 896) -> [P, 4, 2, D] fp8
        wpad = np.zeros((1024, D), np.float32)
        wpad[:D] = wT_scaled
        if bias_row is not None:
            wpad[D] = bias_row
        return np.ascontiguousarray(
            wpad.reshape(4, 2, P, D).transpose(2, 0, 1, 3)).astype(F8NP)

    wa_p = wpack8(Wa.T * SWA, bias_row=ba * SWA)
    wga_p = wpack8(Wg[:, :D].T * SWG)
    wgb_p = wpack8(Wg[:, D:].T * SWG)
    wfT = np.concatenate(
        [Wf[:, :D], 0.5 * Wf[:, D:2 * D], 0.5 * Wf[:, 2 * D:]], axis=1).T
    wf_p = np.ascontiguousarray(
        wfT.reshape(3 * KD, P, D).transpose(1, 0, 2)).astype(BFNP)
    biasp = np.ascontiguousarray(
        np.concatenate([0.5 * bg, bf]).reshape(2 * KD, P).T).astype(np.float32)

    def fmajor(x, i):  # token-major [N, D] -> [P, nch, KD, T] for core i
        blk = x[i * npc:(i + 1) * npc].reshape(nch, T, KD, P)
        return blk.transpose(3, 0, 2, 1)

    n_cores = h.shape[0] // npc
    in_maps = []
    for i in range(n_cores):
        hb = fmajor(h, i)
        ub = fmajor(u, i)
        arr = np.zeros((P, nch, 16, T), np.float32)
        arr[:, :, 0:KD] = hb * SX
        arr[:, :, 8:8 + KD] = ub * SX
        arr[0, :, 15, :] = SX  # constant lane for the L1 bias row
        in_maps.append({
            "xu8": np.ascontiguousarray(arr).astype(F8NP),
            "h16": np.ascontiguousarray(hb).astype(BFNP),
            "u16": np.ascontiguousarray(ub).astype(BFNP),
            "wa8": wa_p, "wga8": wga_p, "wgb8": wgb_p,
            "wf16": wf_p, "biasp": biasp,
        })
    return in_maps


# ---------------------------------------------------------------------------
# legacy all-bf16 path (kept for A/B comparison)

def build_nc(npc=NPC, T=512, mode="bf16", mm_bufs=5, act_bufs=None,
             gelu_native=True):
    if act_bufs is None:
        act_bufs = 2 if mode == "bf16" else 1
    if mode == "bf16":
        cdt = mybir.dt.bfloat16
        mdt = mybir.dt.bfloat16
    elif mode == "fp32r":
        cdt = F32
        mdt = mybir.dt.float32r
    elif mode == "fp32":
        cdt = F32
        mdt = F32
    else:
        raise ValueError(mode)

    n_chunks = npc // T
    assert n_chunks * T == npc

    nc = bacc.Bacc()
    hT_d = nc.declare_dram_parameter("hT", [P, npc // T, KD, T], cdt, isOutput=False)
    uT_d = nc.declare_dram_parameter("uT", [P, npc // T, KD, T], cdt, isOutput=False)
    wa_d = nc.declare_dram_parameter("wa", [P, KD, D], cdt, isOutput=False)
    wg_d = nc.declare_dram_parameter("wg", [P, 2 * KD, D], cdt, isOutput=False)
    wf_d = nc.declare_dram_parameter("wf", [P, 3 * KD, D], cdt, isOutput=False)
    bias_d = nc.declare_dram_parameter("biasp", [P, 3 * KD], F32, isOutput=False)
    gT_d = nc.declare_dram_parameter("gT", [D, npc], F32, isOutput=True)

    inv_sqrt_d = 1.0 / math.sqrt(D)

    def mm(ps, lhsT, rhs, start, stop):
        if mdt != cdt:
            lhsT = lhsT.bitcast(mdt)
            rhs = rhs.bitcast(mdt)
        nc.tensor.matmul(ps, lhsT=lhsT, rhs=rhs, start=start, stop=stop)

    with tile.TileContext(nc) as tc, ExitStack() as ctx:
        wp = ctx.enter_context(tc.tile_pool(name="weights", bufs=1))
        hp = ctx.enter_context(tc.tile_pool(name="hp", bufs=act_bufs))
        up = ctx.enter_context(tc.tile_pool(name="up", bufs=act_bufs))
        uap = ctx.enter_context(tc.tile_pool(name="uap", bufs=act_bufs))
        ugp = ctx.enter_context(tc.tile_pool(name="ugp", bufs=act_bufs))
        hup = ctx.enter_context(tc.tile_pool(name="hup", bufs=act_bufs))
        sp = ctx.enter_context(tc.tile_pool(name="small", bufs=3))
        op = ctx.enter_context(tc.tile_pool(name="outp", bufs=3))
        pp = ctx.enter_context(tc.tile_pool(name="psum", bufs=1, space="PSUM"))

        bias_sb = wp.tile([P, 3 * KD], F32, name="biasp")
        nc.sync.dma_start(bias_sb, bias_d[:, :])
        ones_col = wp.tile([P, 1], cdt, name="ones_col")
        nc.vector.memset(ones_col, 1.0)
        ones_row = wp.tile([1, P], cdt, name="ones_row")
        nc.vector.memset(ones_row, 1.0)

        def load_chunk(c):
            h_sb = hp.tile([P, KD, T], cdt, name=f"h{c}", tag="h")
            nc.sync.dma_start(h_sb, hT_d[:, c])
            u_sb = up.tile([P, KD, T], cdt, name=f"u{c}", tag="u")
            nc.sync.dma_start(u_sb, uT_d[:, c])
            return h_sb, u_sb

        wa_sb = wp.tile([P, KD, D], cdt, name="wa")
        h0_sb = hp.tile([P, KD, T], cdt, name="h0", tag="h")
        u0_sb = up.tile([P, KD, T], cdt, name="u0", tag="u")
        for k in range(KD):
            nc.sync.dma_start(wa_sb[:, k], wa_d[:, k])
            nc.sync.dma_start(u0_sb[:, k], uT_d[:, 0, k])
            nc.sync.dma_start(h0_sb[:, k], hT_d[:, 0, k])
        chunk0 = (h0_sb, u0_sb)
        wg_sb = wp.tile([P, 2 * KD, D], cdt, name="wg")
        nc.sync.dma_start(wg_sb[:, :KD], wg_d[:, :KD])
        nc.sync.dma_start(wg_sb[:, KD:], wg_d[:, KD:])
        wf_sb = wp.tile([P, 3 * KD, D], cdt, name="wf")
        for j in range(3):
            nc.sync.dma_start(wf_sb[:, j * KD:(j + 1) * KD],
                              wf_d[:, j * KD:(j + 1) * KD])

        M_GROUPS = [list(range(0, 4)), list(range(4, KD))]

        for c in range(n_chunks):
            cs = bass.ds(c * T, T)
            h_sb, u_sb = chunk0 if c == 0 else load_chunk(c)

            red = pp.tile([1, T], F32, name=f"red{c}", tag="red", bufs=1)
            tmps = []
            for grp in M_GROUPS:
                pss = {m: pp.tile([P, T], F32, name=f"ps1_{c}_{m}", tag="mm",
                                  bufs=mm_bufs) for m in grp}
                for k in range(KD):
                    for m in grp:
                        mm(pss[m], wa_sb[:, k, m * P:(m + 1) * P], u_sb[:, k, :],
                           start=(k == 0), stop=(k == KD - 1))
                for m in grp:
                    tmp = sp.tile([P, T], cdt, name=f"tmp{c}_{m}", tag="tmp",
                                  bufs=KD)
                    nc.vector.scalar_tensor_tensor(
                        out=tmp, in0=pss[m], scalar=bias_sb[:, m:m + 1],
                        in1=h_sb[:, m, :], op0=ALU.add, op1=ALU.mult)
                    tmps.append(tmp)
            for m in range(KD):
                mm(red, ones_col, tmps[m], start=(m == 0), stop=(m == KD - 1))

            alpha = sp.tile([1, T], cdt, name=f"al{c}", tag="alpha", bufs=2)
            nc.scalar.activation(alpha, red, AF.Tanh, scale=inv_sqrt_d * 0.5)
            ab = pp.tile([P, T], F32, name=f"ab{c}", tag="ab", bufs=2)
            mm(ab, ones_row, alpha, start=True, stop=True)

            ua_sb = uap.tile([P, KD, T], cdt, name=f"ua{c}", tag="ua")
            for k in range(KD):
                nc.vector.scalar_tensor_tensor(
                    out=ua_sb[:, k, :], in0=ab, scalar=1.0, in1=u_sb[:, k, :],
                    op0=ALU.add, op1=ALU.mult)

            ug_sb = ugp.tile([P, KD, T], cdt, name=f"ug{c}", tag="ug")
            for grp in M_GROUPS:
                pss = {m: pp.tile([P, T], F32, name=f"ps2_{c}_{m}", tag="mm",
                                  bufs=mm_bufs) for m in grp}
                for k in range(2 * KD):
                    rhs = h_sb[:, k, :] if k < KD else ua_sb[:, k - KD, :]
                    for m in grp:
                        mm(pss[m], wg_sb[:, k, m * P:(m + 1) * P], rhs,
                           start=(k == 0), stop=(k == 2 * KD - 1))
                for m in grp:
                    t2 = sp.tile([P, T], cdt, name=f"t2_{c}_{m}", tag="t2")
                    nc.scalar.activation(t2, pss[m], AF.Tanh,
                                         bias=bias_sb[:, KD + m:KD + m + 1],
                                         scale=0.5)
                    nc.vector.scalar_tensor_tensor(
                        out=ug_sb[:, m, :], in0=t2, scalar=1.0,
                        in1=ua_sb[:, m, :], op0=ALU.add, op1=ALU.mult)

            hu_sb = hup.tile([P, KD, T], cdt, name=f"hu{c}", tag="hu")
            for k in range(KD):
                nc.vector.tensor_mul(out=hu_sb[:, k, :], in0=h_sb[:, k, :],
                                     in1=ug_sb[:, k, :])

            for m in range(KD):
                ps = pp.tile([P, T], F32, name=f"ps3_{c}_{m}", tag="mm", bufs=mm_bufs)
                for k in range(3 * KD):
                    if k < KD:
                        rhs = h_sb[:, k, :]
                    elif k < 2 * KD:
                        rhs = ug_sb[:, k - KD, :]
                    else:
                        rhs = hu_sb[:, k - 2 * KD, :]
                    mm(ps, wf_sb[:, k, m * P:(m + 1) * P], rhs,
                       start=(k == 0), stop=(k == 3 * KD - 1))
                outp = op.tile([P, T], F32, name=f"o{c}_{m}", tag="out")
                nc.scalar.activation(outp, ps,
                                     AF.Gelu if gelu_native else AF.Identity,
                                     bias=bias_sb[:, 2 * KD + m:2 * KD + m + 1],
                                     scale=1.0)
                nc.scalar.dma_start(gT_d[m * P:(m + 1) * P, cs], outp)
    nc.compile()
    return nc


def prep_inputs(h_t, u_t, W_a_w, W_a_b, W_g_w, W_g_b, W_f_w, W_f_b,
                npc=NPC, T=512, mode="bf16"):
    np_dt = ml_dtypes.bfloat16 if mode == "bf16" else np.float32

    h = np.asarray(h_t, np.float32)
    u = np.asarray(u_t, np.float32)
    Wa = np.asarray(W_a_w, np.float32)
    Wg = np.asarray(W_g_w, np.float32)
    Wf = np.asarray(W_f_w, np.float32)
    ba = np.asarray(W_a_b, np.float32)
    bg = np.asarray(W_g_b, np.float32)
    bf = np.asarray(W_f_b, np.float32)

    waT = Wa.T
    wgT = np.concatenate([Wg[:, :D], Wg[:, D:] * 0.5], axis=1).T
    wfT = np.concatenate([Wf[:, :D], Wf[:, D:2 * D] * 0.25, Wf[:, 2 * D:] * 0.25],
                         axis=1).T

    def wpack(w):
        return np.ascontiguousarray(
            w.reshape(-1, P, D).transpose(1, 0, 2)).astype(np_dt)

    wa_p, wg_p, wf_p = wpack(waT), wpack(wgT), wpack(wfT)
    biasp = np.ascontiguousarray(
        np.concatenate([ba, 0.5 * bg, bf]).reshape(3 * KD, P).T).astype(np.float32)

    nch = npc // T

    def xpack(x, i):
        blk = x[i * npc:(i + 1) * npc]
        blk = blk.reshape(nch, T, KD, P)
        return np.ascontiguousarray(
            blk.transpose(3, 0, 2, 1)).astype(np_dt)

    n_cores = h.shape[0] // npc
    in_maps = []
    for i in range(n_cores):
        in_maps.append({
            "hT": xpack(h, i),
            "uT": xpack(u, i),
            "wa": wa_p, "wg": wg_p, "wf": wf_p, "biasp": biasp,
        })
    return in_maps


_NC_CACHE = {}


def _get_nc(npc=NPC, T=512, mode="fp8"):
    key = (npc, T, mode)
    if key not in _NC_CACHE:
        if mode == "fp8":
            _NC_CACHE[key] = build_nc_fp8(npc=npc, T=T)
        else:
            _NC_CACHE[key] = build_nc(npc=npc, T=T, mode=mode)
    return _NC_CACHE[key]


def run(inputs, npc=NPC, T=None, mode="fp8", trace=False, **kw):
    """Run the SPMD kernel; returns (full_output [N,D] fp32, BassKernelResults)."""
    if T is None:
        T = 512 if mode in ("bf16", "fp8") else 256
    nc = _get_nc(npc=npc, T=T, mode=mode)
    args = (inputs["h_t"], inputs["u_t"], inputs["W_a_w"], inputs["W_a_b"],
            inputs["W_g_w"], inputs["W_g_b"], inputs["W_f_w"], inputs["W_f_b"])
    if mode == "fp8":
        in_maps = prep_inputs_fp8(*args, npc=npc, T=T)
    else:
        in_maps = prep_inputs(*args, npc=npc, T=T, mode=mode)
    res = run_bass_kernel_spmd(nc, in_maps, list(range(len(in_maps))),
                               trace=trace, **kw)
    out = np.concatenate(
        [np.asarray(r["gT"], np.float32).T for r in res.results], axis=0)
    return out, res


def kernel(h_t, u_t, token_idx, u_all, W_a_w, W_a_b, W_g_w, W_g_b, W_f_w, W_f_b):
    # token_idx / u_all are unused by the reference math.
    inputs = {"h_t": h_t, "u_t": u_t, "W_a_w": W_a_w, "W_a_b": W_a_b,
              "W_g_w": W_g_w, "W_g_b": W_g_b, "W_f_w": W_f_w, "W_f_b": W_f_b}
    out, _ = run(inputs)
    return out


if __name__ == "__main__":
    rng = np.random.default_rng(0)
    fake = {
        "h_t": rng.standard_normal((N_TOK, D), dtype=np.float32),
        "u_t": rng.standard_normal((N_TOK, D), dtype=np.float32),
        "W_a_w": rng.standard_normal((D, D), dtype=np.float32) * 0.02,
        "W_a_b": rng.standard_normal((D,), dtype=np.float32) * 0.02,
        "W_g_w": rng.standard_normal((D, 2 * D), dtype=np.float32) * 0.02,
        "W_g_b": rng.standard_normal((D,), dtype=np.float32) * 0.02,
        "W_f_w": rng.standard_normal((D, 3 * D), dtype=np.float32) * 0.02,
        "W_f_b": rng.standard_normal((D,), dtype=np.float32) * 0.02,
    }
    out, res = run(fake)
    print("out", out.shape, out.dtype, "exec_time_ns", res.exec_time_ns)
